# revision 11
# baseline (speedup 1.0000x reference)
"""Trainium2 kernel for nn_DistanceRelativeBias.

Computes out[b,k,i,j] = g_k(||c_i - c_j||) where g_k(d) = b2[k] +
sum_h w2[h,k]*silu(w1[h]*d + b1[h]).

Key ideas vs the previous version (115 us):
  1. SYMMETRY: out[b,k,i,j] == out[b,k,j,i], so only the upper-triangle
     128x128 (i,j) tiles are computed on device (36 of 64 per batch);
     the host mirrors the off-diagonal tiles.  144 tiles / 8 cores = 18
     tiles per core, perfectly balanced.  Tile coordinates are baked
     into per-core constant data (the lhs/rhs columns of the d2 matmul),
     so all cores run one SPMD program.
  2. d^2 DOMAIN: the 16 head-functions are refit (runtime VarPro
     Gauss-Newton, residual ~5e-3) onto an NF-term silu basis in
     u = d^2:  g_k ~= sum_f Q[f,k]*silu(a_f*u + c_f).  No sqrt pass,
     no ACT table switch, no clamp (silu of a slightly-negative u is
     benign, unlike sqrt).
  3. NF=4 basis with 32-row groups: silu cost on ACT scales with
     NF (phi elements), so fewer, wider groups cut ACT work ~2.4x.
     (Falls back to NF=8 / 16-row groups if the runtime fit is poor.)

Per core, per 384-column stripe (3 of its 18 tiles):
  PE  : u[i,j] = -2 c_i.c_j + r_i + r_j   (K=13 hi/lo-split f32r matmul)
  DVE : psum -> fp16 u-tile
  per 32-row group v: PE broadcast-matmul (selector scatters rows across
        partitions p=32f+g with weight a_f) -> ACT silu(.+c_f) -> fp16
        phi -> PE heads matmuls (4x 4-head blocks, m=32k+g) ->
        DVE/ACT cast psum -> fp16 osb
  DMA : osb [128, 6144] fp16 -> DRAM (1.5 MB contiguous, 12KB runs)
Host unscrambles (pure data movement) + mirrors + upcasts to fp32.
"""
import numpy as np

B, N, D = 4, 1024, 3
HID, HEADS = 64, 16
NCORES = 8
NT = 18                 # 128x128 tiles per core
NSB = 3                 # tile-subblocks per stripe
NST = NT // NSB         # stripes per core (6)
W = NSB * 128           # stripe width (384)
_KD = 13                # d2 matmul contraction (hi/lo split for f32r)
_AMAX = 24.0            # silu slope bound (fp16-robustness of the basis)

# upper-triangle tile list per batch: 36 tiles; cores 2b, 2b+1 take halves
TILES = [(ci, cj) for ci in range(8) for cj in range(ci, 8)]

_prog_cache = {}


def _round10(x):
    """Round mantissa to 10 bits (conservative f32r grid)."""
    i = np.asarray(x, np.float32).view(np.int32).astype(np.int64)
    r = (i + 0x1000 + ((i >> 13) & 1)) >> 13 << 13
    return (r & 0xFFFFFFFF).astype(np.uint32).view(np.float32)


def _silu(x):
    x = np.clip(x, -60.0, 60.0)
    return x / (1.0 + np.exp(-x))


# ---------------------------------------------------------------- basis fit
def _g_exact(d, w1, b1, w2, b2):
    return _silu(d[..., None] * w1 + b1) @ w2 + b2


def _solveQ(usamp, Gs, gnorm, a, c, lam=1e-10):
    phi = _silu(usamp[:, None] * a + c)
    A = phi.T @ phi + lam * np.eye(a.size)
    try:
        Q = np.linalg.solve(A, phi.T @ Gs)
    except np.linalg.LinAlgError:
        Q = np.linalg.lstsq(phi, Gs, rcond=None)[0]
    return Q, np.linalg.norm(phi @ Q - Gs) / gnorm


def _varpro(usamp, Gs, gnorm, a0, c0, iters=80):
    """Variable-projection Gauss-Newton over (a, c); Q solved exactly."""
    a, c = a0.astype(np.float64).copy(), c0.astype(np.float64).copy()
    NF = a.size
    Q, f = _solveQ(usamp, Gs, gnorm, a, c)
    lm = 1e-3
    for _ in range(iters):
        p = np.concatenate([a, c])
        r0 = (_silu(usamp[:, None] * a + c) @ Q - Gs).ravel()
        J = np.empty((r0.size, 2 * NF))
        for i in range(2 * NF):
            dp = np.zeros(2 * NF)
            dp[i] = max(1e-5, 1e-6 * abs(p[i]))
            a2 = np.clip((p + dp)[:NF], -_AMAX, _AMAX)
            c2 = (p + dp)[NF:]
            Q2, _ = _solveQ(usamp, Gs, gnorm, a2, c2)
            J[:, i] = ((_silu(usamp[:, None] * a2 + c2) @ Q2 - Gs).ravel() - r0) / dp[i]
        JtJ = J.T @ J
        Jtr = J.T @ r0
        ok = False
        for _ in range(10):
            try:
                step = np.linalg.solve(
                    JtJ + lm * np.diag(np.maximum(np.diag(JtJ), 1e-12)), -Jtr)
            except np.linalg.LinAlgError:
                lm *= 10
                continue
            a2 = np.clip(a + step[:NF], -_AMAX, _AMAX)
            c2 = c + step[NF:]
            Q2, f2 = _solveQ(usamp, Gs, gnorm, a2, c2)
            if f2 < f:
                a, c, Q, f = a2, c2, Q2, f2
                lm = max(lm * 0.3, 1e-8)
                ok = True
                break
            lm *= 10
        if not ok or lm > 1e9:
            break
    return a, c, Q, f


def _fit_basis(coords, w1, b1, w2, b2, NF):
    """Fit g_k(sqrt(u)) ~= sum_f Q[f,k] silu(a_f u + c_f) over the actual
    pairwise-d^2 distribution.  Returns (a fp16-snapped, c, Q, sim_rel)."""
    c64 = coords.astype(np.float64)
    w1 = w1.astype(np.float64).reshape(-1)
    b1 = b1.astype(np.float64)
    w2 = w2.astype(np.float64)
    b2 = b2.astype(np.float64)
    us = []
    for b in range(B):
        cb = c64[b]
        r = (cb * cb).sum(1)
        us.append(np.maximum(r[:, None] + r[None, :] - 2.0 * (cb @ cb.T), 0.0).ravel())
    uall = np.concatenate(us)
    nq = 8192
    usamp = np.quantile(uall, (np.arange(nq) + 0.5) / nq)
    usamp = np.concatenate([usamp, np.zeros(nq // 1024)])  # diagonal at true mass
    Gs = _g_exact(np.sqrt(usamp), w1, b1, w2, b2)
    gnorm = np.linalg.norm(Gs)

    inits = {
        4: [(np.array([0.186, -0.132, 0.519, -1.03]), np.array([2., -2., -1., -14.])),
            (np.array([0.3, -0.3, 1.0, -2.0]), np.array([1., -1., -3., 2.]))],
        8: [(np.array([0.186, -0.132, 0.519, -1.03, 0.024, -0.731, -2.428, -13.477]),
             np.array([2., -2., -1., -14., -2., 1., -14., -14.]))],
    }[NF]
    best = None
    for a0, c0 in inits:
        a, c, Q, f = _varpro(usamp, Gs, gnorm, a0, c0)
        aq = np.float16(a).astype(np.float64)
        Q2, f2 = _solveQ(usamp, Gs, gnorm, aq, c)
        if best is None or f2 < best[0]:
            best = (f2, aq, c, Q2)
    _, a, c, Q = best

    # fp16 end-to-end simulation on batch 0 + all diagonals (norm-weighted)
    u0 = us[0]
    Gt = _g_exact(np.sqrt(u0), w1, b1, w2, b2)
    xq = np.float16(u0).astype(np.float64)
    phi = np.float16(_silu(xq[:, None] * a + c)).astype(np.float64)
    Gf = np.float16(phi @ np.float16(Q).astype(np.float64)).astype(np.float64)
    rel = np.linalg.norm(Gf - Gt) / np.linalg.norm(Gt)
    return a, c, Q, rel


# ------------------------------------------------------------- device data
def _pack_core(coords_b, tlist, avec, cvec, Q, GSZ):
    """Per-core constant tensors for one batch-half (18 tiles)."""
    NF = avec.size
    NGRP = 128 // GSZ
    NQ = 16 * GSZ // 128
    KH = 16 // NQ

    c64 = coords_b.astype(np.float64)
    cf = c64.astype(np.float32)
    ch = _round10(cf)
    cl = (cf.astype(np.float64) - ch).astype(np.float32)
    r = (c64 * c64).sum(1)
    rf = r.astype(np.float32)
    rh = _round10(rf)
    rl = (rf.astype(np.float64) - rh).astype(np.float32)

    # cstf: [rhs blocks (18x128)] then [scatter-lhs blocks (18x4x128)]
    # scatter-lhs col p carries the i-row GSZ*v + (p % GSZ), so the d2
    # matmul directly produces u replicated NF times across partitions
    cstf = np.zeros((_KD, (1 + NGRP) * NT * 128), np.float32)
    p = np.arange(128)
    for t, (ci, cj) in enumerate(tlist):
        si = slice(128 * ci, 128 * ci + 128)
        sj = slice(128 * cj, 128 * cj + 128)
        L = np.zeros((_KD, 128), np.float32)
        L[0:3] = -2.0 * ch[si].T
        L[3:6] = -2.0 * ch[si].T
        L[6:9] = -2.0 * cl[si].T
        L[9] = rh[si]
        L[10] = rl[si]
        L[11] = 1.0
        L[12] = 1.0
        R = np.zeros((_KD, 128), np.float32)
        R[0:3] = ch[sj].T
        R[3:6] = cl[sj].T
        R[6:9] = ch[sj].T
        R[9] = 1.0
        R[10] = 1.0
        R[11] = rh[sj]
        R[12] = rl[sj]
        cstf[:, 128 * t:128 * (t + 1)] = R
        for v in range(NGRP):
            o = NT * 128 + (t * NGRP + v) * 128
            cstf[:, o:o + 128] = L[:, GSZ * v + (p % GSZ)]

    cst16 = np.zeros((128, NQ * 128), np.float32)
    for q in range(NQ):
        Qm = np.zeros((128, 128), np.float32)
        for f in range(NF):
            for kh in range(KH):
                for g in range(GSZ):
                    Qm[GSZ * f + g, GSZ * kh + g] = Q[f, KH * q + kh]
        cst16[:, 128 * q:128 * (q + 1)] = Qm
    cst16 = cst16.astype(np.float16)

    cstv = np.stack([cvec[p // GSZ], avec[p // GSZ]], axis=1).astype(np.float32)
    return {"cstf": cstf, "cst16": cst16, "cstv": cstv}


# ---------------------------------------------------------------- program
def _build_program(GSZ):
    import concourse.bacc as bacc
    import concourse.mybir as mybir
    import concourse.tile as tile

    f32 = mybir.dt.float32
    f32r = mybir.dt.float32r
    f16 = mybir.dt.float16
    AF = mybir.ActivationFunctionType

    NGRP = 128 // GSZ
    NQ = 16 * GSZ // 128
    GW = NQ * W                # heads-psum cols per group
    OC = NGRP * GW             # osb cols per stripe
    OUTCOLS = NST * OC

    nc = bacc.Bacc(num_devices=NCORES)
    CSTF = nc.declare_dram_parameter("cstf", [_KD, (1 + NGRP) * NT * 128], f32r, isOutput=False)
    CST16 = nc.declare_dram_parameter("cst16", [128, NQ * 128], f16, isOutput=False)
    CSTV = nc.declare_dram_parameter("cstv", [128, 2], f32, isOutput=False)
    OUT = nc.declare_dram_parameter("out", [128, OUTCOLS], f16, isOutput=True)

    # out-cast engine split: ACT handles ~9/24 of the psum->sbuf casts
    ncast = NST * NGRP
    nact = max(1, (9 * ncast) // 24)
    pat = []
    acc = 0.0
    for _ in range(ncast):
        acc += nact / ncast
        if acc >= 1.0:
            pat.append('A')
            acc -= 1.0
        else:
            pat.append('D')

    with tile.TileContext(nc) as tc:
        with (
            tc.tile_pool(name="const", bufs=1) as cp,
            tc.tile_pool(name="phi", bufs=4) as php,
            tc.tile_pool(name="osb", bufs=2) as osp,
            tc.tile_pool(name="psA", bufs=2, space="PSUM") as psA,
            tc.tile_pool(name="psB", bufs=2, space="PSUM") as psB,
        ):
            ddr = cp.tile([_KD, (1 + NGRP) * NT * 128], f32r, tag="ddr")
            nc.sync.dma_start(ddr[:], CSTF[:])
            c16 = cp.tile([128, NQ * 128], f16, tag="c16")
            nc.scalar.dma_start(c16[:], CST16[:])
            cv = cp.tile([128, 2], f32, tag="cv")
            nc.scalar.dma_start(cv[:], CSTV[:])

            # silu table load + PE HAM warmup while constants stream in
            warm = cp.tile([128, W], f16, tag="warm")
            nc.vector.memset(warm[:], 0.0)
            warm2 = cp.tile([128, 8], f32, tag="warm2")
            nc.vector.memset(warm2[:], 0.0)
            warm3 = cp.tile([128, 8], f32, tag="warm3")
            nc.scalar.activation(warm3[:], warm2[:], AF.Silu)
            pw = psA.tile([128, W], f32, tag="ps")
            for _ in range(10):
                nc.tensor.matmul(pw[:], warm[:, 0:128], warm[:],
                                 start=True, stop=True)

            # software-pipelined main loop: the scatter-d2 matmul of group
            # g+1 is issued to the PE before heads of group g, so the PE
            # never waits on the ACT silu; the silu applies the per-
            # partition slope (scale AP) and offset (bias AP) directly to
            # the replicated-u psum; osb halves DMA out as casts land
            NG = NST * NGRP
            osbt = [None] * NST
            pht = [None] * NG
            ndma = 0
            for g in range(NG + 1):
                if g < NG:
                    s, v = divmod(g, NGRP)
                    if v == 0:
                        osb = osp.tile([128, OC], f16, tag="osb")
                        osbt[s] = osb
                    pph = psA.tile([128, W], f32, tag="ps")
                    for u3 in range(NSB):
                        t = NSB * s + u3
                        o = NT * 128 + (t * NGRP + v) * 128
                        nc.tensor.matmul(
                            pph[:, 128 * u3:128 * (u3 + 1)],
                            ddr[:, o:o + 128],
                            ddr[:, 128 * t:128 * (t + 1)],
                            start=True, stop=True)
                    phi = php.tile([128, W], f16, tag="phi")
                    nc.scalar.activation(phi[:], pph[:], AF.Silu,
                                         bias=cv[:, 0:1], scale=cv[:, 1:2])
                    pht[g] = phi
                if g >= 1:
                    h = g - 1
                    sh, vh = divmod(h, NGRP)
                    po = psB.tile([128, GW], f32, tag="po")
                    for q in range(NQ):
                        # split at PSUM bank boundaries: one matmul output
                        # must stay within a single 512-col bank
                        x0 = W * q
                        while x0 < W * (q + 1):
                            x1 = min(W * (q + 1), (x0 // 512 + 1) * 512)
                            nc.tensor.matmul(
                                po[:, x0:x1],
                                c16[:, 128 * q:128 * (q + 1)],
                                pht[h][:, x0 - W * q:x1 - W * q],
                                start=True, stop=True)
                            x0 = x1
                    dst = osbt[sh][:, GW * vh:GW * (vh + 1)]
                    if pat[h] == 'A':
                        nc.scalar.copy(dst, po[:])
                    else:
                        nc.vector.tensor_copy(dst, po[:])
                    if vh == NGRP // 2 - 1 or vh == NGRP - 1:
                        half = 0 if vh < NGRP // 2 else 1
                        hw = OC // 2
                        eng = (nc.gpsimd, nc.sync)[ndma % 2]
                        ndma += 1
                        eng.dma_start(
                            OUT[:, OC * sh + hw * half:OC * sh + hw * (half + 1)],
                            osbt[sh][:, hw * half:hw * (half + 1)])
    nc.compile()
    return nc


# -------------------------------------------------------------------- run
def _run(coords, w1, b1, w2, b2, trace=False):
    from concourse.bass_utils import run_bass_kernel_spmd

    coords = np.asarray(coords)
    avec, cvec, Q, rel = _fit_basis(coords, w1, b1, w2, b2, 4)
    GSZ = 32
    if rel > 1.2e-2:   # fallback: richer basis, 16-row groups
        avec, cvec, Q, rel = _fit_basis(coords, w1, b1, w2, b2, 8)
        GSZ = 16

    if GSZ not in _prog_cache:
        _prog_cache[GSZ] = _build_program(GSZ)
    nc = _prog_cache[GSZ]

    in_maps = []
    for core in range(NCORES):
        b, half = divmod(core, 2)
        tlist = TILES[18 * half:18 * (half + 1)]
        in_maps.append(_pack_core(coords[b], tlist, avec, cvec, Q, GSZ))

    res = run_bass_kernel_spmd(nc, in_maps, list(range(NCORES)), trace=trace)

    NGRP = 128 // GSZ
    NQ = 16 * GSZ // 128
    KH = 16 // NQ
    out = np.empty((B, HEADS, N, N), dtype=np.float32)
    for core in range(NCORES):
        b, half = divmod(core, 2)
        tlist = TILES[18 * half:18 * (half + 1)]
        raw = res.results[core]["out"]
        # [m, col] -> [kh, g, s, v, q, u, jj]
        A = raw.reshape(KH, GSZ, NST, NGRP, NQ, NSB, 128)
        for t, (ci, cj) in enumerate(tlist):
            s, u3 = divmod(t, NSB)
            blk = A[:, :, s, :, :, u3, :]            # [kh, g, v, q, jj]
            tl = blk.transpose(3, 0, 2, 1, 4).reshape(HEADS, 128, 128)
            i0, j0 = 128 * ci, 128 * cj
            out[b, :, i0:i0 + 128, j0:j0 + 128] = tl
            if ci != cj:
                out[b, :, j0:j0 + 128, i0:i0 + 128] = tl.transpose(0, 2, 1)
    return out, res


def kernel(coords, w1, b1, w2, b2):
    out, _ = _run(coords, w1, b1, w2, b2, trace=False)
    return out


# revision 17
# speedup vs baseline: 1.0797x; 1.0797x over previous
"""Trainium2 kernel for nn_DistanceRelativeBias.

Computes out[b,k,i,j] = g_k(||c_i - c_j||) where g_k(d) = b2[k] +
sum_h w2[h,k]*silu(w1[h]*d + b1[h]).

Key ideas vs the previous version (115 us):
  1. SYMMETRY: out[b,k,i,j] == out[b,k,j,i], so only the upper-triangle
     128x128 (i,j) tiles are computed on device (36 of 64 per batch);
     the host mirrors the off-diagonal tiles.  144 tiles / 8 cores = 18
     tiles per core, perfectly balanced.  Tile coordinates are baked
     into per-core constant data (the lhs/rhs columns of the d2 matmul),
     so all cores run one SPMD program.
  2. d^2 DOMAIN: the 16 head-functions are refit (runtime VarPro
     Gauss-Newton, residual ~5e-3) onto an NF-term silu basis in
     u = d^2:  g_k ~= sum_f Q[f,k]*silu(a_f*u + c_f).  No sqrt pass,
     no ACT table switch, no clamp (silu of a slightly-negative u is
     benign, unlike sqrt).
  3. NF=4 basis with 32-row groups: silu cost on ACT scales with
     NF (phi elements), so fewer, wider groups cut ACT work ~2.4x.
     (Falls back to NF=8 / 16-row groups if the runtime fit is poor.)

Per core, per 384-column stripe (3 of its 18 tiles):
  PE  : u[i,j] = -2 c_i.c_j + r_i + r_j   (K=13 hi/lo-split f32r matmul)
  DVE : psum -> fp16 u-tile
  per 32-row group v: PE broadcast-matmul (selector scatters rows across
        partitions p=32f+g with weight a_f) -> ACT silu(.+c_f) -> fp16
        phi -> PE heads matmuls (4x 4-head blocks, m=32k+g) ->
        DVE/ACT cast psum -> fp16 osb
  DMA : osb [128, 6144] fp16 -> DRAM (1.5 MB contiguous, 12KB runs)
Host unscrambles (pure data movement) + mirrors + upcasts to fp32.
"""
import numpy as np

B, N, D = 4, 1024, 3
HID, HEADS = 64, 16
NCORES = 8
NT = 18                 # 128x128 tiles per core
NSB = 3                 # tile-subblocks per stripe
NST = NT // NSB         # stripes per core (6)
W = NSB * 128           # stripe width (384)
_KD = 13                # d2 matmul contraction (hi/lo split for f32r)
_AMAX = 24.0            # silu slope bound (fp16-robustness of the basis)

# upper-triangle tile list per batch: 36 tiles; cores 2b, 2b+1 take halves
TILES = [(ci, cj) for ci in range(8) for cj in range(ci, 8)]

_prog_cache = {}


def _round10(x):
    """Round mantissa to 10 bits (conservative f32r grid)."""
    i = np.asarray(x, np.float32).view(np.int32).astype(np.int64)
    r = (i + 0x1000 + ((i >> 13) & 1)) >> 13 << 13
    return (r & 0xFFFFFFFF).astype(np.uint32).view(np.float32)


def _silu(x):
    x = np.clip(x, -60.0, 60.0)
    return x / (1.0 + np.exp(-x))


# ---------------------------------------------------------------- basis fit
def _g_exact(d, w1, b1, w2, b2):
    return _silu(d[..., None] * w1 + b1) @ w2 + b2


def _solveQ(usamp, Gs, gnorm, a, c, lam=1e-10):
    phi = _silu(usamp[:, None] * a + c)
    A = phi.T @ phi + lam * np.eye(a.size)
    try:
        Q = np.linalg.solve(A, phi.T @ Gs)
    except np.linalg.LinAlgError:
        Q = np.linalg.lstsq(phi, Gs, rcond=None)[0]
    return Q, np.linalg.norm(phi @ Q - Gs) / gnorm


def _varpro(usamp, Gs, gnorm, a0, c0, iters=80):
    """Variable-projection Gauss-Newton over (a, c); Q solved exactly."""
    a, c = a0.astype(np.float64).copy(), c0.astype(np.float64).copy()
    NF = a.size
    Q, f = _solveQ(usamp, Gs, gnorm, a, c)
    lm = 1e-3
    for _ in range(iters):
        p = np.concatenate([a, c])
        r0 = (_silu(usamp[:, None] * a + c) @ Q - Gs).ravel()
        J = np.empty((r0.size, 2 * NF))
        for i in range(2 * NF):
            dp = np.zeros(2 * NF)
            dp[i] = max(1e-5, 1e-6 * abs(p[i]))
            a2 = np.clip((p + dp)[:NF], -_AMAX, _AMAX)
            c2 = (p + dp)[NF:]
            Q2, _ = _solveQ(usamp, Gs, gnorm, a2, c2)
            J[:, i] = ((_silu(usamp[:, None] * a2 + c2) @ Q2 - Gs).ravel() - r0) / dp[i]
        JtJ = J.T @ J
        Jtr = J.T @ r0
        ok = False
        for _ in range(10):
            try:
                step = np.linalg.solve(
                    JtJ + lm * np.diag(np.maximum(np.diag(JtJ), 1e-12)), -Jtr)
            except np.linalg.LinAlgError:
                lm *= 10
                continue
            a2 = np.clip(a + step[:NF], -_AMAX, _AMAX)
            c2 = c + step[NF:]
            Q2, f2 = _solveQ(usamp, Gs, gnorm, a2, c2)
            if f2 < f:
                a, c, Q, f = a2, c2, Q2, f2
                lm = max(lm * 0.3, 1e-8)
                ok = True
                break
            lm *= 10
        if not ok or lm > 1e9:
            break
    return a, c, Q, f


def _fit_basis(coords, w1, b1, w2, b2, NF):
    """Fit g_k(sqrt(u)) ~= sum_f Q[f,k] silu(a_f u + c_f) over the actual
    pairwise-d^2 distribution.  Returns (a fp16-snapped, c, Q, sim_rel)."""
    c64 = coords.astype(np.float64)
    w1 = w1.astype(np.float64).reshape(-1)
    b1 = b1.astype(np.float64)
    w2 = w2.astype(np.float64)
    b2 = b2.astype(np.float64)
    us = []
    for b in range(B):
        cb = c64[b]
        r = (cb * cb).sum(1)
        us.append(np.maximum(r[:, None] + r[None, :] - 2.0 * (cb @ cb.T), 0.0).ravel())
    uall = np.concatenate(us)
    nq = 8192
    usamp = np.quantile(uall, (np.arange(nq) + 0.5) / nq)
    usamp = np.concatenate([usamp, np.zeros(nq // 1024)])  # diagonal at true mass
    Gs = _g_exact(np.sqrt(usamp), w1, b1, w2, b2)
    gnorm = np.linalg.norm(Gs)

    inits = {
        4: [(np.array([0.186, -0.132, 0.519, -1.03]), np.array([2., -2., -1., -14.])),
            (np.array([0.3, -0.3, 1.0, -2.0]), np.array([1., -1., -3., 2.]))],
        8: [(np.array([0.186, -0.132, 0.519, -1.03, 0.024, -0.731, -2.428, -13.477]),
             np.array([2., -2., -1., -14., -2., 1., -14., -14.]))],
    }[NF]
    best = None
    for a0, c0 in inits:
        a, c, Q, f = _varpro(usamp, Gs, gnorm, a0, c0)
        aq = np.float16(a).astype(np.float64)
        Q2, f2 = _solveQ(usamp, Gs, gnorm, aq, c)
        if best is None or f2 < best[0]:
            best = (f2, aq, c, Q2)
    _, a, c, Q = best

    # fp16 end-to-end simulation on batch 0 + all diagonals (norm-weighted)
    u0 = us[0]
    Gt = _g_exact(np.sqrt(u0), w1, b1, w2, b2)
    xq = np.float16(u0).astype(np.float64)
    phi = np.float16(_silu(xq[:, None] * a + c)).astype(np.float64)
    Gf = np.float16(phi @ np.float16(Q).astype(np.float64)).astype(np.float64)
    rel = np.linalg.norm(Gf - Gt) / np.linalg.norm(Gt)
    return a, c, Q, rel


# ------------------------------------------------------------- device data
def _pack_core(coords_b, tlist, avec, cvec, Q, GSZ):
    """Per-core constant tensors for one batch-half (18 tiles)."""
    NF = avec.size
    NGRP = 128 // GSZ
    NQ = 16 * GSZ // 128
    KH = 16 // NQ

    c64 = coords_b.astype(np.float64)
    cf = c64.astype(np.float32)
    ch = _round10(cf)
    cl = (cf.astype(np.float64) - ch).astype(np.float32)
    r = (c64 * c64).sum(1)
    rf = r.astype(np.float32)
    rh = _round10(rf)
    rl = (rf.astype(np.float64) - rh).astype(np.float32)

    cstf = np.zeros((_KD, 2 * NT * 128), np.float32)
    for t, (ci, cj) in enumerate(tlist):
        si = slice(128 * ci, 128 * ci + 128)
        sj = slice(128 * cj, 128 * cj + 128)
        L = np.zeros((_KD, 128), np.float32)
        L[0:3] = -2.0 * ch[si].T
        L[3:6] = -2.0 * ch[si].T
        L[6:9] = -2.0 * cl[si].T
        L[9] = rh[si]
        L[10] = rl[si]
        L[11] = 1.0
        L[12] = 1.0
        R = np.zeros((_KD, 128), np.float32)
        R[0:3] = ch[sj].T
        R[3:6] = cl[sj].T
        R[6:9] = ch[sj].T
        R[9] = 1.0
        R[10] = 1.0
        R[11] = rh[sj]
        R[12] = rl[sj]
        cstf[:, 128 * t:128 * (t + 1)] = L
        cstf[:, NT * 128 + 128 * t:NT * 128 + 128 * (t + 1)] = R

    cst16 = np.zeros((128, (NGRP + NQ) * 128), np.float32)
    p = np.arange(128)
    a_perm = avec[p // GSZ]
    for v in range(NGRP):
        S = np.zeros((128, 128), np.float32)
        S[GSZ * v + (p % GSZ), p] = a_perm
        cst16[:, 128 * v:128 * (v + 1)] = S
    for q in range(NQ):
        Qm = np.zeros((128, 128), np.float32)
        for f in range(NF):
            for kh in range(KH):
                for g in range(GSZ):
                    Qm[GSZ * f + g, GSZ * kh + g] = Q[f, KH * q + kh]
        cst16[:, 128 * (NGRP + q):128 * (NGRP + q + 1)] = Qm
    cst16 = cst16.astype(np.float16)

    cstv = cvec[p // GSZ].astype(np.float32).reshape(128, 1)
    return {"cstf": cstf, "cst16": cst16, "cstv": cstv}


# ---------------------------------------------------------------- program
def _build_program(GSZ):
    import concourse.bacc as bacc
    import concourse.mybir as mybir
    import concourse.tile as tile

    f32 = mybir.dt.float32
    f32r = mybir.dt.float32r
    f16 = mybir.dt.float16
    AF = mybir.ActivationFunctionType

    NGRP = 128 // GSZ
    NQ = 16 * GSZ // 128
    GW = NQ * W                # heads-psum cols per group
    OC = NGRP * GW             # osb cols per stripe
    OUTCOLS = NST * OC

    nc = bacc.Bacc(num_devices=NCORES)
    CSTF = nc.declare_dram_parameter("cstf", [_KD, 2 * NT * 128], f32r, isOutput=False)
    CST16 = nc.declare_dram_parameter("cst16", [128, (NGRP + NQ) * 128], f16, isOutput=False)
    CSTV = nc.declare_dram_parameter("cstv", [128, 1], f32, isOutput=False)
    OUT = nc.declare_dram_parameter("out", [128, OUTCOLS], f16, isOutput=True)

    # out-cast engine split: ACT handles ~10/24 of the psum->sbuf casts
    ncast = NST * NGRP
    nact = max(1, (10 * ncast) // 24)
    pat = []
    acc = 0.0
    for _ in range(ncast):
        acc += nact / ncast
        if acc >= 1.0:
            pat.append('A')
            acc -= 1.0
        else:
            pat.append('D')

    with tile.TileContext(nc) as tc:
        with (
            tc.tile_pool(name="const", bufs=1) as cp,
            tc.tile_pool(name="ut", bufs=2) as utp,
            tc.tile_pool(name="phi", bufs=4) as php,
            tc.tile_pool(name="osb", bufs=2) as osp,
            tc.tile_pool(name="psA", bufs=2, space="PSUM") as psA,
            tc.tile_pool(name="psB", bufs=2, space="PSUM") as psB,
        ):
            ddr = cp.tile([_KD, 2 * NT * 128], f32r, tag="ddr")
            nc.sync.dma_start(ddr[:], CSTF[:])
            c16 = cp.tile([128, (NGRP + NQ) * 128], f16, tag="c16")
            nc.scalar.dma_start(c16[:], CST16[:])
            cv = cp.tile([128, 1], f32, tag="cv")
            nc.scalar.dma_start(cv[:], CSTV[:])

            # silu table load + PE HAM warmup while constants stream in
            warm = cp.tile([128, W], f16, tag="warm")
            nc.vector.memset(warm[:], 0.0)
            warm2 = cp.tile([128, 8], f32, tag="warm2")
            nc.vector.memset(warm2[:], 0.0)
            warm3 = cp.tile([128, 8], f32, tag="warm3")
            nc.scalar.activation(warm3[:], warm2[:], AF.Silu)
            pw = psA.tile([128, W], f32, tag="ps")
            for _ in range(10):
                nc.tensor.matmul(pw[:], warm[:, 0:128], warm[:],
                                 start=True, stop=True)

            # software-pipelined main loop: broadcast of group g+1 is issued
            # to the PE before heads of group g, so the PE never waits on the
            # ACT silu; osb halves DMA out as soon as their casts land
            NG = NST * NGRP
            utt = [None] * NST
            osbt = [None] * NST
            pht = [None] * NG
            ndma = 0
            for g in range(NG + 1):
                if g < NG:
                    s, v = divmod(g, NGRP)
                    if v == 0:
                        pd = psA.tile([128, W], f32, tag="ps")
                        for u3 in range(NSB):
                            t = NSB * s + u3
                            nc.tensor.matmul(
                                pd[:, 128 * u3:128 * (u3 + 1)],
                                ddr[:, 128 * t:128 * (t + 1)],
                                ddr[:, NT * 128 + 128 * t:NT * 128 + 128 * (t + 1)],
                                start=True, stop=True)
                        ut = utp.tile([128, W], f16, tag="ut")
                        nc.vector.tensor_copy(ut[:], pd[:])
                        utt[s] = ut
                        osb = osp.tile([128, OC], f16, tag="osb")
                        osbt[s] = osb
                    pph = psA.tile([128, W], f32, tag="ps")
                    nc.tensor.matmul(pph[:], c16[:, 128 * v:128 * (v + 1)],
                                     utt[s][:], start=True, stop=True)
                    phi = php.tile([128, W], f16, tag="phi")
                    nc.scalar.activation(phi[:], pph[:], AF.Silu,
                                         bias=cv[:, 0:1], scale=1.0)
                    pht[g] = phi
                if g >= 1:
                    h = g - 1
                    sh, vh = divmod(h, NGRP)
                    po = psB.tile([128, GW], f32, tag="po")
                    for q in range(NQ):
                        # split at PSUM bank boundaries: one matmul output
                        # must stay within a single 512-col bank
                        x0 = W * q
                        while x0 < W * (q + 1):
                            x1 = min(W * (q + 1), (x0 // 512 + 1) * 512)
                            nc.tensor.matmul(
                                po[:, x0:x1],
                                c16[:, 128 * (NGRP + q):128 * (NGRP + q + 1)],
                                pht[h][:, x0 - W * q:x1 - W * q],
                                start=True, stop=True)
                            x0 = x1
                    dst = osbt[sh][:, GW * vh:GW * (vh + 1)]
                    if pat[h] == 'A':
                        nc.scalar.copy(dst, po[:])
                    else:
                        nc.vector.tensor_copy(dst, po[:])
                    if vh == NGRP // 2 - 1 or vh == NGRP - 1:
                        half = 0 if vh < NGRP // 2 else 1
                        hw = OC // 2
                        eng = (nc.gpsimd, nc.sync)[ndma % 2]
                        ndma += 1
                        eng.dma_start(
                            OUT[:, OC * sh + hw * half:OC * sh + hw * (half + 1)],
                            osbt[sh][:, hw * half:hw * (half + 1)])
    nc.compile()
    return nc


# -------------------------------------------------------------------- run
def _run(coords, w1, b1, w2, b2, trace=False):
    from concourse.bass_utils import run_bass_kernel_spmd

    coords = np.asarray(coords)
    avec, cvec, Q, rel = _fit_basis(coords, w1, b1, w2, b2, 4)
    GSZ = 32
    if rel > 1.2e-2:   # fallback: richer basis, 16-row groups
        avec, cvec, Q, rel = _fit_basis(coords, w1, b1, w2, b2, 8)
        GSZ = 16

    if GSZ not in _prog_cache:
        _prog_cache[GSZ] = _build_program(GSZ)
    nc = _prog_cache[GSZ]

    in_maps = []
    for core in range(NCORES):
        b, half = divmod(core, 2)
        tlist = TILES[18 * half:18 * (half + 1)]
        in_maps.append(_pack_core(coords[b], tlist, avec, cvec, Q, GSZ))

    res = run_bass_kernel_spmd(nc, in_maps, list(range(NCORES)), trace=trace)

    NGRP = 128 // GSZ
    NQ = 16 * GSZ // 128
    KH = 16 // NQ
    out = np.empty((B, HEADS, N, N), dtype=np.float32)
    for core in range(NCORES):
        b, half = divmod(core, 2)
        tlist = TILES[18 * half:18 * (half + 1)]
        raw = res.results[core]["out"]
        # [m, col] -> [kh, g, s, v, q, u, jj]
        A = raw.reshape(KH, GSZ, NST, NGRP, NQ, NSB, 128)
        for t, (ci, cj) in enumerate(tlist):
            s, u3 = divmod(t, NSB)
            blk = A[:, :, s, :, :, u3, :]            # [kh, g, v, q, jj]
            tl = blk.transpose(3, 0, 2, 1, 4).reshape(HEADS, 128, 128)
            i0, j0 = 128 * ci, 128 * cj
            out[b, :, i0:i0 + 128, j0:j0 + 128] = tl
            if ci != cj:
                out[b, :, j0:j0 + 128, i0:i0 + 128] = tl.transpose(0, 2, 1)
    return out, res


def kernel(coords, w1, b1, w2, b2):
    out, _ = _run(coords, w1, b1, w2, b2, trace=False)
    return out
